# revision 28
# baseline (speedup 1.0000x reference)
"""DBRX-style MoE layer on 8 TRN2 NeuronCores — expert-parallel.

Sharding: expert e lives on core e (w1_v1[e], w2[e] transposed host-side).
x and the gate are replicated. Each core computes the router exactly (fp32),
compacts the token list routed to its expert on-device (top-64 per 128-token
row-group via DVE max8/match_replace rounds; token id + routing weight packed
into one fp32), gathers those token rows of x (indirect DMA, bf16),
PE-transposes them, runs the GLU MLP (bf16 matmuls, fp32 accumulate), scales
rows by the routing weight, and returns (vals[C,H], idx[C], w[C]). The host
scatter-adds the 8 sparse shards into the full [T, H] output (the unshard).

Each compaction round emits one 128-slot c-block, so gather/transpose/MM1
pipeline against the remaining rounds.

Self-contained: hardcodes all shapes from the problem spec.
"""

import os
import sys

# recover gracefully if a previous process left the cores wedged
os.environ.setdefault("NEURON_RT_RESET_CORES", "1")

for _p in ("/opt/trn_rl_repo", "/root/.axon_site/_ro/trn_rl_repo"):
    if os.path.isdir(_p) and _p not in sys.path:
        sys.path.append(_p)

import numpy as np
import ml_dtypes

import concourse.bass as bass
import concourse.mybir as mybir
import concourse.tile as tile
from concourse.bass import IndirectOffsetOnAxis
from concourse.bass_utils import run_bass_kernel_spmd

T, H, F, E = 2048, 1024, 1024, 8
P = 128
C = 896          # capacity: 16 row-groups x 56 slots (row mean 32, sigma 4.9)
CB = C // P      # 8 c-blocks == compaction rounds
TC = T // P      # 16 token tiles
HC = H // P      # 8 h-chunks
FC = F // P      # 8 f-chunks
F32 = mybir.dt.float32
BF16 = mybir.dt.bfloat16
I32 = mybir.dt.int32
AF = mybir.ActivationFunctionType
ALU = mybir.AluOpType
AX = mybir.AxisListType

_wait_ctr = [0]


def _split_attached_waits(nc):
    """This walrus rejects instruction-attached sem waits on compute/DMA
    structs; re-encode them as standalone single-wait EventSemaphores (the
    raw-bass wait_ge encoding, which compiles and runs)."""
    for f in nc.m.functions:
        for bb in f.blocks:
            new = []
            for inst in bb.instructions:
                si = inst.sync_info
                waits = list(si.on_wait) if si is not None else []
                is_ev = inst.opcode == "EventSemaphore"
                if waits and not (is_ev and len(waits) == 1):
                    keep = []
                    if is_ev:
                        keep, waits = waits[:1], waits[1:]
                    for w in waits:
                        _wait_ctr[0] += 1
                        ev = mybir.InstEventSemaphore(
                            name=f"waitsplit_{_wait_ctr[0]}", ins=[], outs=[]
                        )
                        ev.engine = inst.engine
                        ev.sync_info = mybir.SyncInfo(on_wait=[w], on_update=[])
                        new.append(ev)
                    inst.sync_info = mybir.SyncInfo(
                        on_wait=keep, on_update=list(si.on_update)
                    )
                new.append(inst)
            bb.instructions = new


def build():
    nc = bass.Bass()

    xT_d = nc.dram_tensor("xT", [H, T], F32, kind="ExternalInput")
    xb_d = nc.dram_tensor("xb", [T, H], BF16, kind="ExternalInput")
    gT_d = nc.dram_tensor("gT", [H, E], F32, kind="ExternalInput")
    oh_d = nc.dram_tensor("oh", [P, TC * E], F32, kind="ExternalInput")
    id_d = nc.dram_tensor("idm", [P, P], F32, kind="ExternalInput")
    idb_d = nc.dram_tensor("idmb", [P, P], BF16, kind="ExternalInput")
    w1_d = nc.dram_tensor("w1t", [H, 2 * F], BF16, kind="ExternalInput")
    w2_d = nc.dram_tensor("w2t", [F, H], BF16, kind="ExternalInput")

    vals_d = nc.dram_tensor("vals", [C, H], BF16, kind="ExternalOutput")
    idx_d = nc.dram_tensor("idx", [C], I32, kind="ExternalOutput")
    wred_d = nc.dram_tensor("wred", [C], F32, kind="ExternalOutput")
    warm_d = nc.dram_tensor("warm", [1, 8], F32)
    warm2_d = nc.dram_tensor("warm2", [1, 8], F32)

    with tile.TileContext(nc) as tc:
        with (
            tc.tile_pool(name="const", bufs=1) as constp,
            tc.tile_pool(name="big", bufs=1) as bigp,
            tc.tile_pool(name="xts", bufs=2) as xtp,
            tc.tile_pool(name="xgs", bufs=6) as xgp,
            tc.tile_pool(name="work", bufs=1) as workp,
            tc.tile_pool(name="outs", bufs=3) as outp,
        ):
            # ---- router-critical loads first, then resident weights ------
            id128 = constp.tile([P, P], F32, tag="id128")
            nc.sync.dma_start(id128[:], id_d[:])
            id128b = constp.tile([P, P], BF16, tag="id128b")
            nc.sync.dma_start(id128b[:], idb_d[:])
            gate = constp.tile([P, HC, E], F32, tag="gate")
            nc.sync.dma_start(
                gate[:], gT_d[:].rearrange("(hc p) e -> p hc e", p=P)
            )
            xts_all = []
            for i in range(T // 512):
                xts = xtp.tile([P, HC, 512], F32, tag=f"xts{i % 2}")
                for g in range(2):
                    nc.sync.dma_start(
                        xts[:, g * 4 : (g + 1) * 4, :],
                        xT_d[
                            g * 4 * P : (g + 1) * 4 * P, i * 512 : (i + 1) * 512
                        ].rearrange("(c p) t -> p c t", p=P),
                    )
                xts_all.append(xts)
            ohb = constp.tile([P, TC * E], F32, tag="ohb")
            nc.sync.dma_start(ohb[:], oh_d[:])
            w1sb = bigp.tile([P, HC, 2 * F], BF16, tag="w1sb")
            logits = workp.tile([P, TC * E], F32, tag="logits")
            lgT = workp.tile([E, T], F32, tag="lgT")
            a1p = workp.tile([P, TC], F32, tag="a1p")
            a1 = workp.tile([16, P], F32, tag="a1")
            with tc.tile_pool(name="psA", bufs=2, space="PSUM") as psA:
                # PE warmup while router inputs stream in (HAM needs ~3.4us
                # of array-busy time to unlock 2.4 GHz)
                wps = psA.tile([P, 512], F32, tag="warmps")
                for r in range(8):
                    nc.tensor.matmul(
                        wps[:, :P], id128[:], id128[:], start=(r == 0), stop=(r == 7)
                    )
                wsb = workp.tile([1, 8], F32, tag="warmsb")
                nc.vector.tensor_copy(wsb[:], wps[:1, :8])
                nc.sync.dma_start(warm_d[:], wsb[:])

                # ---- router: logits exact fp32, gate stationary ----------
                max8 = workp.tile([P, TC * E], F32, tag="max8")
                exps = workp.tile([P, TC * E], F32, tag="exps")
                sums = workp.tile([P, TC], F32, tag="sums")
                tmp = workp.tile([P, TC * E], F32, tag="tmp")
                lcol = workp.tile([P, TC], F32, tag="lcol")
                ecol = workp.tile([P, TC], F32, tag="ecol")
                for i in range(T // 512):
                    lgp = psA.tile([E, 512], F32, tag="lgp")
                    for hc in range(HC):
                        nc.tensor.matmul(
                            lgp[:],
                            gate[:, hc, :],
                            xts_all[i][:, hc, :],
                            start=(hc == 0),
                            stop=(hc == HC - 1),
                        )
                    nc.vector.tensor_copy(lgT[:, i * 512 : (i + 1) * 512], lgp[:])
                    for l in range(4):
                        tt = i * 4 + l
                        tpl = psA.tile([P, E], F32, tag="tpl")
                        nc.tensor.transpose(
                            tpl[:], lgT[:, tt * P : (tt + 1) * P], id128[:E, :E]
                        )
                        nc.vector.tensor_copy(
                            logits[:, tt * E : (tt + 1) * E], tpl[:]
                        )
                        nc.vector.max(
                            max8[:, tt * E : (tt + 1) * E],
                            logits[:, tt * E : (tt + 1) * E],
                        )
                    csl = slice(i * 4 * E, (i + 1) * 4 * E)
                    c4 = slice(i * 4, (i + 1) * 4)
                    nc.scalar.activation(exps[:, csl], logits[:, csl], AF.Exp)
                    nc.vector.tensor_reduce(
                        sums[:, c4],
                        exps[:, csl].rearrange("p (a b) -> p a b", b=E),
                        axis=AX.X, op=ALU.add,
                    )
                    nc.vector.tensor_mul(tmp[:, csl], logits[:, csl], ohb[:, csl])
                    nc.vector.tensor_reduce(
                        lcol[:, c4],
                        tmp[:, csl].rearrange("p (a b) -> p a b", b=E),
                        axis=AX.X, op=ALU.add,
                    )
                    nc.vector.tensor_mul(tmp[:, csl], exps[:, csl], ohb[:, csl])
                    nc.vector.tensor_reduce(
                        ecol[:, c4],
                        tmp[:, csl].rearrange("p (a b) -> p a b", b=E),
                        axis=AX.X, op=ALU.add,
                    )

                nc.sync.dma_start(
                    w1sb[:], w1_d[:].rearrange("(hc p) m -> p hc m", p=P)
                )
                rcp = workp.tile([P, TC], F32, tag="rcp")
                nc.vector.reciprocal(rcp[:], sums[:])

                m2 = max8[:].rearrange("p (a b) -> p a b", b=E)[:, :, 1]
                sel = workp.tile([P, TC], F32, tag="sel")
                nc.vector.tensor_tensor(out=sel[:], in0=lcol[:], in1=m2, op=ALU.is_ge)
                comb = workp.tile([P, TC], F32, tag="comb")
                nc.vector.tensor_mul(comb[:], ecol[:], rcp[:])
                nc.vector.tensor_mul(comb[:], comb[:], sel[:])

                # compaction input: selected -> id + weight, else -1
                ids = workp.tile([P, TC], F32, tag="ids")
                nc.gpsimd.iota(
                    ids[:], pattern=[[P, TC]], channel_multiplier=1,
                    allow_small_or_imprecise_dtypes=True,
                )
                isel = workp.tile([P, TC], F32, tag="isel")
                nc.vector.tensor_scalar_add(isel[:], ids[:], 1.0)
                nc.vector.tensor_mul(isel[:], isel[:], sel[:])
                nc.vector.tensor_scalar_add(isel[:], isel[:], -1.0)
                nc.vector.tensor_add(a1p[:], isel[:], comb[:])

                tp = psA.tile([16, P], F32, tag="tps")
                nc.tensor.transpose(tp[:], a1p[:], id128[:])
                nc.vector.tensor_copy(a1[:], tp[:])

                # anchored PE warmup: bridges the HAM through the DVE-only
                # compaction window; reads a1p so the a1 rounds aren't blocked
                wps2 = psA.tile([16, 512], F32, tag="warmps")
                for r in range(18):
                    nc.tensor.matmul(
                        wps2[:], a1p[:], xts_all[3][:, 0, :],
                        start=(r == 0), stop=(r == 17),
                    )
                wsb2 = workp.tile([1, 8], F32, tag="warmsb")
                nc.vector.tensor_copy(wsb2[:], wps2[:1, :8])
                nc.sync.dma_start(warm2_d[:], wsb2[:])

            # ---- compaction rounds, one c-block per round ----------------
            # per round: only the gather-critical idx column; weight
            # extraction is deferred to one batched pass after the rounds
            CR = C // 16
            m1 = workp.tile([16, CR], F32, tag="m1")
            mh = workp.tile([16, CR], F32, tag="mh")
            iraw = workp.tile([16, CR], I32, tag="iraw")
            icl = workp.tile([16, CR], I32, tag="icl")
            idxs = constp.tile([P, CB], I32, tag="idxs")
            wcol = constp.tile([P, CB], F32, tag="wcol")
            for b in range(CB):
                sl = slice(b * 8, (b + 1) * 8)
                nc.vector.max(m1[:, sl], a1[:])
                if b < CB - 1:
                    nc.vector.match_replace(
                        out=a1[:], in_to_replace=m1[:, sl],
                        in_values=a1[:], imm_value=-2.0,
                    )
                # id = nearest-int(val - 0.5)  [cast rounds to nearest even]
                nc.vector.tensor_scalar_add(mh[:, sl], m1[:, sl], -0.5)
                nc.vector.tensor_copy(iraw[:, sl], mh[:, sl])
                nc.vector.tensor_scalar_max(icl[:, sl], iraw[:, sl], 0)
                nc.sync.dma_start(idxs[:, b : b + 1], icl[:, sl])

            w2sb = bigp.tile([P, FC, H], BF16, tag="w2sb")
            nc.sync.dma_start(
                w2sb[:], w2_d[:].rearrange("(fc p) h -> p fc h", p=P)
            )

            # batched weight extraction: w = (val - id) masked to 0 on
            # invalid (-1/-2) slots
            ifl = workp.tile([16, CR], F32, tag="ifl")
            maskv = workp.tile([16, CR], F32, tag="maskv")
            wfin = workp.tile([16, CR], F32, tag="wfin")
            nc.vector.tensor_copy(ifl[:], iraw[:])
            nc.vector.tensor_scalar(maskv[:], m1[:], 0.0, None, op0=ALU.is_ge)
            nc.vector.tensor_sub(wfin[:], m1[:], ifl[:])
            nc.vector.tensor_mul(wfin[:], wfin[:], maskv[:])
            for b in range(CB):
                nc.sync.dma_start(
                    wcol[:, b : b + 1], wfin[:, b * 8 : (b + 1) * 8]
                )

            # ---- gather (bf16) -> PE transpose -> MM1, pipelined ---------
            xgT = bigp.tile([P, HC, C], BF16, tag="xgT")
            hid = bigp.tile([P, FC, C], BF16, tag="hid")

            def transpose_block(psum_pool, xg, b):
                for hc in range(HC):
                    tp2 = psum_pool.tile([P, P], BF16, tag="tp2")
                    nc.tensor.transpose(
                        tp2[:], xg[:, hc * P : (hc + 1) * P], id128b[:]
                    )
                    nc.vector.tensor_copy(
                        xgT[:, hc, b * P : (b + 1) * P], tp2[:]
                    )

            def mm1_block(psum_pool, fb, cstart, cn):
                pg = psum_pool.tile([P, 512], F32, tag="pg")
                pv = psum_pool.tile([P, 512], F32, tag="pv")
                for hc in range(HC):
                    nc.tensor.matmul(
                        pg[:, :cn], w1sb[:, hc, fb * P : (fb + 1) * P],
                        xgT[:, hc, cstart : cstart + cn],
                        start=(hc == 0), stop=(hc == HC - 1),
                    )
                for hc in range(HC):
                    nc.tensor.matmul(
                        pv[:, :cn], w1sb[:, hc, F + fb * P : F + (fb + 1) * P],
                        xgT[:, hc, cstart : cstart + cn],
                        start=(hc == 0), stop=(hc == HC - 1),
                    )
                sg = outp.tile([P, 512], BF16, tag="sg")
                nc.scalar.activation(sg[:, :cn], pg[:, :cn], AF.Silu)
                nc.vector.tensor_mul(
                    hid[:, fb, cstart : cstart + cn], sg[:, :cn], pv[:, :cn]
                )

            with tc.tile_pool(name="psB", bufs=2, space="PSUM") as psB:
                xgs = []
                for b in range(CB):
                    xg = xgp.tile([P, H], BF16, tag="xg")
                    nc.gpsimd.indirect_dma_start(
                        out=xg[:],
                        out_offset=None,
                        in_=xb_d[:],
                        in_offset=IndirectOffsetOnAxis(
                            ap=idxs[:, b : b + 1], axis=0
                        ),
                    )
                    xgs.append(xg)
                for b in range(4):
                    transpose_block(psB, xgs[b], b)
                mm1_block(psB, 0, 0, 512)
                for b in range(4, CB):
                    transpose_block(psB, xgs[b], b)
                for fb in range(1, FC):
                    mm1_block(psB, fb, 0, 512)
                for fb in range(FC):
                    mm1_block(psB, fb, 512, C - 512)

                # ---- MM2: out[c, h] = hiddenT.T @ w2T, scale, store ------
                for cb in range(CB):
                    for hh in range(2):
                        po = psB.tile([P, 512], F32, tag="po")
                        for fc in range(FC):
                            nc.tensor.matmul(
                                po[:],
                                hid[:, fc, cb * P : (cb + 1) * P],
                                w2sb[:, fc, hh * 512 : (hh + 1) * 512],
                                start=(fc == 0), stop=(fc == FC - 1),
                            )
                        ot = outp.tile([P, 512], BF16, tag="ot")
                        nc.vector.tensor_scalar_mul(ot[:], po[:], wcol[:, cb : cb + 1])
                        nc.sync.dma_start(
                            vals_d[cb * P : (cb + 1) * P, hh * 512 : (hh + 1) * 512],
                            ot[:],
                        )

            # idx/w external outputs (off the critical path); global row
            # order is 128*b + 8*r + j, matching the per-round columns
            nc.sync.dma_start(
                idx_d[:].rearrange("(b r j) -> r b j", r=16, j=8),
                icl[:].rearrange("r (b j) -> r b j", j=8),
            )
            nc.sync.dma_start(
                wred_d[:].rearrange("(b r j) -> r b j", r=16, j=8),
                wfin[:].rearrange("r (b j) -> r b j", j=8),
            )

    _split_attached_waits(nc)
    return nc


_NC = None


def _get_nc():
    global _NC
    if _NC is None:
        _NC = build()
    return _NC


def kernel(x, gate_w, w1_v1, w2, _trace=False):
    x = np.ascontiguousarray(np.asarray(x, dtype=np.float32))
    gate_w = np.ascontiguousarray(np.asarray(gate_w, dtype=np.float32))
    w1_v1 = np.ascontiguousarray(np.asarray(w1_v1, dtype=np.float32))
    w2 = np.ascontiguousarray(np.asarray(w2, dtype=np.float32))

    xT = np.ascontiguousarray(x.T)
    xb = x.astype(ml_dtypes.bfloat16)
    gT = np.ascontiguousarray(gate_w.T)
    eye = np.eye(E, dtype=np.float32)
    idm = np.eye(P, dtype=np.float32)
    in_maps = []
    for e in range(E):
        in_maps.append(
            {
                "xT": xT,
                "xb": xb,
                "gT": gT,
                "oh": np.ascontiguousarray(
                    np.tile(np.tile(eye[e], TC)[None, :], (P, 1))
                ),
                "idm": idm,
                "idmb": idm.astype(ml_dtypes.bfloat16),
                "w1t": np.ascontiguousarray(w1_v1[e].T).astype(ml_dtypes.bfloat16),
                "w2t": np.ascontiguousarray(w2[e].T).astype(ml_dtypes.bfloat16),
            }
        )

    nc = _get_nc()
    res = run_bass_kernel_spmd(nc, in_maps, list(range(E)), trace=_trace)
    kernel.last_exec_time_ns = res.exec_time_ns

    out = np.zeros((T, H), dtype=np.float32)
    for e in range(E):
        r = res.results[e]
        vals = np.asarray(r["vals"], dtype=np.float32)
        idx = np.asarray(r["idx"]).astype(np.int64)
        w = np.asarray(r["wred"], dtype=np.float32)
        m = (w > 0) & (idx >= 0) & (idx < T)
        out[idx[m]] += vals[m]
    return out


kernel.last_exec_time_ns = None
